# revision 25
# baseline (speedup 1.0000x reference)
"""Multi-head attention (B=2, S=2048, H=1024, NH=16, HD=64) on 8 trn2 cores.

Sharding: tensor-parallel over heads. Core c owns heads {2c, 2c+1}, i.e.
feature columns [128c, 128c+128) of q/k/v. Wq/Wk/Wv are column-sharded,
Wo row-sharded; each core computes a full-shape partial output and the
host sums the 8 partials (the row-parallel reduce) during unshard.

On-chip layout is feature-major ("transposed"): the host passes
hsT = hidden_states.T so both matmul operands of every projection have
the contraction dim on partitions and no on-chip transposes of big
tensors are needed. Attention works on scoresT[tk, tq]; softmax's
normalizer comes from a ones-column augmented V matmul (exp is safe
without max-subtraction because scores are O(6) here).

QKV and output projections run in float32r (fast fp32 mode, ~1.4e-4 rel
accuracy); score/ctx matmul operands are fp16; all accumulation is full
fp32 in PSUM. Attention matmuls are zero-padded to full 128x128 array
shapes (K=128 scores via zero-padded per-head K, M=128 ctx via padded
augmented-V) - half-array matmuls don't register as activity for the
PE's HAM clock gate and the whole phase runs at 1.2GHz otherwise.
"""

import numpy as np

B, S, H, NH, HD = 2, 2048, 1024, 16, 64
NCORES = 8
JC = 128  # head-columns per core (2 heads x 64)
T = B * S  # 4096 tokens
TQB = 512  # tq block
NKT = S // 128  # 16 tk blocks per batch
WAVE = 1024  # qkv projection token-chunk per wave
BASE = 10000.0

_nc_cache = [None]

_LDW_OPT = False


def _patch_ldw_opt():
    from concourse import bass_utils as _bu

    if getattr(_bu, "_ldw_patched", False):
        return
    _orig = _bu.run_command

    def _patched(argv, **kw):
        argv = [
            a.replace("--enable-ldw-opt=false", "--enable-ldw-opt=true")
            if _LDW_OPT and isinstance(a, str)
            else a
            for a in argv
        ]
        return _orig(argv, **kw)

    _bu.run_command = _patched
    _bu._ldw_patched = True


def _build():
    _patch_ldw_opt()
    import concourse.tile as tile
    from concourse import bacc, mybir
    from concourse.masks import make_identity

    F32 = mybir.dt.float32
    F32R = mybir.dt.float32r
    BF16 = mybir.dt.bfloat16
    F16 = mybir.dt.float16
    EXP = mybir.ActivationFunctionType.Exp

    nc = bacc.Bacc("TRN2", target_bir_lowering=False, debug=False)

    hsT = nc.dram_tensor("hsT", [H, T], F32R, kind="ExternalInput").ap()
    wqT = nc.dram_tensor("wqT", [H, JC], F32R, kind="ExternalInput").ap()
    wkT = nc.dram_tensor("wkT", [H, JC], F32R, kind="ExternalInput").ap()
    wvT = nc.dram_tensor("wvT", [H, JC], F32R, kind="ExternalInput").ap()
    woJI = nc.dram_tensor("woJI", [JC, H], F32R, kind="ExternalInput").ap()
    cosT = nc.dram_tensor("cosT", [128, S], F32, kind="ExternalInput").ap()
    sinTs = nc.dram_tensor("sinTs", [128, S], F32, kind="ExternalInput").ap()
    out = nc.dram_tensor("out", [T, H], F32, kind="ExternalOutput").ap()

    with tile.TileContext(nc) as tc:
        with (
            tc.tile_pool(name="wts", bufs=1) as wts,
            tc.tile_pool(name="tabs", bufs=1) as tabs,
            tc.tile_pool(name="hst", bufs=20) as hst,
            tc.tile_pool(name="qkv", bufs=2) as qkvp,
            tc.tile_pool(name="ps", bufs=3, space="PSUM") as ps,
            tc.tile_pool(name="cxp", bufs=2, space="PSUM") as cxp,
            tc.tile_pool(name="rope", bufs=3) as ropep,
            tc.tile_pool(name="vaug", bufs=1) as vaugp,
            tc.tile_pool(name="expt", bufs=4) as exptp,
            tc.tile_pool(name="ctx", bufs=1) as ctxp,
            tc.tile_pool(name="nrm", bufs=3) as nrmp,
            tc.tile_pool(name="outs", bufs=3) as outsp,
            tc.tile_pool(name="zdr", bufs=4, space="DRAM") as zdrp,
        ):
            # ---- persistent weights / tables ----
            wq_sb = wts.tile([128, 8, JC], F32R, tag="wq")
            nc.sync.dma_start(
                out=wq_sb[:], in_=wqT[:, :].rearrange("(k p) j -> p k j", p=128)
            )
            wk_sb = wts.tile([128, 8, JC], F32R, tag="wk")
            nc.sync.dma_start(
                out=wk_sb[:], in_=wkT[:, :].rearrange("(k p) j -> p k j", p=128)
            )
            wv_sb = wts.tile([128, 8, JC], F32R, tag="wv")
            nc.sync.dma_start(
                out=wv_sb[:], in_=wvT[:, :].rearrange("(k p) j -> p k j", p=128)
            )
            wJ = wts.tile([128, H], F32R, tag="wj")
            nc.sync.dma_start(out=wJ[:], in_=woJI[:, :])
            cos_sb = tabs.tile([128, S], F32, tag="cos")
            nc.sync.dma_start(out=cos_sb[:], in_=cosT[:, :])
            sin_sb = tabs.tile([128, S], F32, tag="sin")
            nc.sync.dma_start(out=sin_sb[:], in_=sinTs[:, :])
            ident = tabs.tile([128, 128], F32, tag="ident")
            make_identity(nc, ident[:])
            onesc = tabs.tile([128, NKT], F32, tag="ones")
            nc.vector.memset(onesc[:], 1.0)

            for b in range(B):
                # ======== QKV projections (+RoPE), feature-major ========
                qT = qkvp.tile([128, S], F16, tag="qT")
                kT = qkvp.tile([128, S], F16, tag="kT")
                vT = qkvp.tile([128, S], F32, tag="vT")

                chains = []
                for nchi in range(S // TQB):
                    for kind, w_sb in (("q", wq_sb), ("k", wk_sb), ("v", wv_sb)):
                        chains.append((kind, w_sb, nchi))
                chunk_cache = {}

                def get_chunk(k, nchi):
                    if (k, nchi) not in chunk_cache:
                        t0 = b * S + nchi * TQB
                        c = hst.tile([128, TQB], F32R, tag="hst")
                        nc.sync.dma_start(
                            out=c[:], in_=hsT[128 * k : 128 * (k + 1), t0 : t0 + TQB]
                        )
                        chunk_cache[(k, nchi)] = c
                    return chunk_cache[(k, nchi)]

                for i0 in range(0, len(chains), 3):
                    pair = chains[i0 : i0 + 3]
                    pt_a = ps.tile([128, TQB], F32, tag="ps")
                    pt_b = ps.tile([128, TQB], F32, tag="ps")
                    pt_c = ps.tile([128, TQB], F32, tag="ps")
                    ptiles = [pt_a, pt_b, pt_c][: len(pair)]
                    for k in range(8):
                        for (kind, w_sb, nchi), p in zip(pair, ptiles):
                            nc.tensor.matmul(
                                p[:], w_sb[:, k, :], get_chunk(k, nchi)[:],
                                start=(k == 0), stop=(k == 7),
                            )
                    for (kind, w_sb, nchi), p in zip(pair, ptiles):
                        sl = slice(nchi * TQB, (nchi + 1) * TQB)
                        if kind == "v":
                            nc.vector.tensor_copy(vT[:, sl], p[:])
                            continue
                        dstT = qT if kind == "q" else kT
                        raw = ropep.tile([128, TQB], F32, tag="raw")
                        nc.vector.tensor_copy(raw[:], p[:])
                        rot = ropep.tile([128, TQB], F32, tag="rot")
                        for h0 in (0, 64):
                            nc.sync.dma_start(
                                out=rot[h0 : h0 + 32, :], in_=raw[h0 + 32 : h0 + 64, :]
                            )
                            nc.sync.dma_start(
                                out=rot[h0 + 32 : h0 + 64, :], in_=raw[h0 : h0 + 32, :]
                            )
                        t1 = ropep.tile([128, TQB], F32, tag="t1")
                        nc.vector.tensor_mul(t1[:], raw[:], cos_sb[:, sl])
                        t2 = ropep.tile([128, TQB], F32, tag="t2")
                        nc.vector.tensor_mul(t2[:], rot[:], sin_sb[:, sl])
                        nc.vector.tensor_add(dstT[:, sl], t1[:], t2[:])

                # zero-padded per-head K so scores run full-array K=128
                kZA = qkvp.tile([128, S], F16, tag="kZA")
                nc.vector.memset(kZA[64:128, :], 0.0)
                nc.vector.tensor_copy(kZA[0:64, :], kT[0:64, :])
                kZB = qkvp.tile([128, S], F16, tag="kZB")
                nc.vector.memset(kZB[0:64, :], 0.0)
                nc.vector.tensor_copy(kZB[64:128, :], kT[64:128, :])

                # ======== v transpose -> per-head augmented V (M padded to 128) ====
                vA = vaugp.tile([128, NKT, 128], F16, tag="vA")
                vB = vaugp.tile([128, NKT, 128], F16, tag="vB")
                nc.vector.memset(vA[:, :, 65:128], 0.0)
                nc.vector.memset(vB[:, :, 65:128], 0.0)
                nc.vector.tensor_copy(vA[:, :, 64], onesc[:])
                nc.vector.tensor_copy(vB[:, :, 64], onesc[:])
                for tkb in range(NKT):
                    pt = ps.tile([128, WAVE], F32, tag="ps")
                    nc.tensor.transpose(
                        pt[:, 0:128], vT[:, 128 * tkb : 128 * (tkb + 1)], ident[:]
                    )
                    nc.vector.tensor_copy(vA[:, tkb, 0:64], pt[:, 0:64])
                    nc.vector.tensor_copy(vB[:, tkb, 0:64], pt[:, 64:128])

                # ======== attention: scoresT -> exp -> ctxT ========
                ctxS = ctxp.tile([128, S], F32R, tag="cts")
                ctxB = ctxp.tile([64, S], F32R, tag="ctb")
                ctxA = ctxS
                for tqb in range(S // TQB):
                    qsl = slice(tqb * TQB, (tqb + 1) * TQB)
                    cxA = cxp.tile([128, TQB], F32, tag="cx")
                    cxB = cxp.tile([128, TQB], F32, tag="cx")
                    for p in range(NKT // 2):
                        scA = ps.tile([128, 2 * TQB], F32, tag="ps")
                        scB = ps.tile([128, 2 * TQB], F32, tag="ps")
                        for t in range(2):
                            tkb = 2 * p + t
                            ksl = slice(128 * tkb, 128 * (tkb + 1))
                            nc.tensor.matmul(
                                scA[:, t * TQB : (t + 1) * TQB],
                                kZA[:, ksl], qT[:, qsl],
                                start=True, stop=True,
                            )
                            nc.tensor.matmul(
                                scB[:, t * TQB : (t + 1) * TQB],
                                kZB[:, ksl], qT[:, qsl],
                                start=True, stop=True,
                            )
                        etA = exptp.tile([128, 2 * TQB], F16, tag="et")
                        nc.scalar.activation(etA[:], scA[:], EXP, scale=0.125)
                        etB = exptp.tile([128, 2 * TQB], F16, tag="et")
                        nc.scalar.activation(etB[:], scB[:], EXP, scale=0.125)
                        for t in range(2):
                            tkb = 2 * p + t
                            st, sp = tkb == 0, tkb == NKT - 1
                            tsl = slice(t * TQB, (t + 1) * TQB)
                            nc.tensor.matmul(
                                cxA[:, :], vA[:, tkb, :], etA[:, tsl],
                                start=st, stop=sp,
                            )
                            nc.tensor.matmul(
                                cxB[:, :], vB[:, tkb, :], etB[:, tsl],
                                start=st, stop=sp,
                            )
                    for cx, ctxT in ((cxA, ctxS), (cxB, ctxB)):
                        craw = nrmp.tile([65, TQB], F32, tag="craw")
                        nc.vector.tensor_copy(craw[:], cx[0:65, :])
                        rzf = nrmp.tile([1, TQB], F32, tag="rzf")
                        nc.vector.reciprocal(rzf[:], craw[64:65, :])
                        zd = zdrp.tile([1, TQB], F32, tag="zd")
                        nc.sync.dma_start(out=zd[:], in_=rzf[:])
                        zrep = nrmp.tile([64, TQB], F32, tag="zrep")
                        nc.sync.dma_start(
                            out=zrep[:], in_=zd[0:1, :].to_broadcast([64, TQB])
                        )
                        dst = ctxT[0:64, qsl] if ctxT is ctxS else ctxT[:, qsl]
                        nc.vector.tensor_mul(dst, craw[0:64, :], zrep[:])
                        if ctxT is ctxB:
                            nc.sync.dma_start(
                                out=ctxS[64:128, qsl], in_=ctxB[:, qsl]
                            )

                # ======== output projection (natural-layout out) ========
                for tq8 in range(S // 128):
                    po = ps.tile([128, WAVE], F32, tag="ps")
                    csl = slice(128 * tq8, 128 * (tq8 + 1))
                    for ich in range(2):
                        isl = slice(ich * 512, (ich + 1) * 512)
                        nc.tensor.matmul(
                            po[:, isl], ctxS[:, csl], wJ[:, isl], start=True, stop=True
                        )
                    ot = outsp.tile([128, H], F32, tag="ot")
                    if tq8 % 2 == 0:
                        nc.vector.tensor_copy(ot[:], po[:])
                    else:
                        nc.scalar.copy(ot[:], po[:])
                    nc.sync.dma_start(
                        out=out[b * S + 128 * tq8 : b * S + 128 * (tq8 + 1), :],
                        in_=ot[:],
                    )

    nc.compile()
    return nc


def _rope_tables():
    inv_freq = 1.0 / (BASE ** (np.arange(0, HD, 2, dtype=np.float64) / HD))
    t = np.arange(S, dtype=np.float64)
    freqs = np.outer(t, inv_freq)  # [S, 32]
    emb = np.concatenate([freqs, freqs], -1)  # [S, 64]
    cos = np.cos(emb).T.astype(np.float32)  # [64, S]
    sin = np.sin(emb).T.astype(np.float32)
    sin_signed = sin.copy()
    sin_signed[0:32] = -sin_signed[0:32]
    cosT = np.ascontiguousarray(np.tile(cos, (2, 1)))  # [128, S]
    sinTs = np.ascontiguousarray(np.tile(sin_signed, (2, 1)))
    return cosT, sinTs


def kernel(hidden_states, Wq, Wk, Wv, Wo):
    hidden_states = np.asarray(hidden_states, np.float32)
    Wq, Wk, Wv, Wo = (np.asarray(w, np.float32) for w in (Wq, Wk, Wv, Wo))

    if _nc_cache[0] is None:
        _nc_cache[0] = _build()
    nc = _nc_cache[0]

    hsT = np.ascontiguousarray(hidden_states.reshape(T, H).T)  # [H, T]
    cosT, sinTs = _rope_tables()
    in_maps = []
    for c in range(NCORES):
        sl = slice(JC * c, JC * (c + 1))
        in_maps.append(
            {
                "hsT": hsT,
                "wqT": np.ascontiguousarray(Wq[sl, :].T),
                "wkT": np.ascontiguousarray(Wk[sl, :].T),
                "wvT": np.ascontiguousarray(Wv[sl, :].T),
                "woJI": np.ascontiguousarray(Wo[:, sl].T),
                "cosT": cosT,
                "sinTs": sinTs,
            }
        )

    from concourse.bass_utils import run_bass_kernel_spmd

    res = run_bass_kernel_spmd(nc, in_maps, core_ids=list(range(NCORES)))
    acc = np.zeros((T, H), np.float64)
    for c in range(NCORES):
        acc += res.results[c]["out"]
    return acc.astype(np.float32).reshape(B, S, H)
